# revision 25
# baseline (speedup 1.0000x reference)
"""Trainium2 Bass kernel for the ergodicity loss (Chebyshev-cascade scheme).

Math: for x[T=512, B=16, N=32, d=2] in [0,1]^2 and modes (k0,k1) in {0..9}^2:
    basis = cos(pi*k0*x0) * cos(pi*k1*x1)                    (separable)
    coeffs[b, k0, k1] = sum_{t,n} basis / (T*N) / nf[k1]
    loss = mean((nw * (coeffs - cd))**2)

Device strategy (8 cores, data-parallel over T: 64 timesteps/core):
  Per core: 2048 points per batch as 128 partitions x 16 chunks, both
  coordinate dims side by side (xx[p, dd*256 + c*16 + b]).

  x is staged host-side as fp16 (x in [0,1): 2^-12 rounding error, far
  below the loss tolerance) to halve the input HBM traffic.

  Mode columns (bf16) hold, per mode k, a KNOWN LINEAR COMBINATION of
  the true cosines c_j = cos(pi*j*x), built WITHOUT any range-reduction
  tricks via a Chebyshev product cascade seeded by one direct Sin:
    - p1 = Sin(-pi*x + pi/2) = cos(pi*x) = c1   (arg in (-pi/2, pi/2],
      inside the HW Sin table domain; ACT, reads xx fp16 directly)
    - p2 = Square(sqrt2*p1) = 2*c1^2 = 1 + c2             (ACT)
    - p4 = p2*p2                                          (Pool TT)
    - [p6,p8] = broadcast(p4) * [p2,p4]                   (one DVE TT)
    - [p3,p5,p7,p9] = [p2,p4,p6,p8] * broadcast(p1)       (one DVE TT)
      (stride-16 plane APs + stride-0 broadcast sub-columns)
    - p0 = 1                                              (Pool memset)
  The mode-k columns are thus M @ c for a constant triangular M
  (|Minv|max ~ 100; high-mode noise amplification is crushed by the
  k^-3 loss weights nw); the PSUM Gram matrix S~ = M S M^T is unmixed
  on the host with M^-1 (exact linear algebra, fp64).

  PE: per (chunk c in 16, batch-group g in 2): lhsT = dim-0 columns
  [k:10 x b:8] (contiguous 80-col block), rhs = dim-1 columns;
  accumulate into one PSUM[80, 160] tile over the 16 chunks — BOTH
  groups share one 2KB bank (g=0's start=True zeroes the whole bank
  before g=1's first start=False accumulate; skip_group_check bypasses
  the one-group-per-bank guard), 6-deep rotation. Off-diagonal batch
  blocks unused. One ACT copy evacuates PSUM to SBUF, one DMA out.

  The activation-table load (1283 ns) is hoisted out of the bench loop
  by a pre-loop warmup Sin whose (zero) output feeds the bias tile
  every in-loop activation consumes (Sin/Square/Copy share one HW
  activation table set, so no in-loop reloads occur).
Host: sum 8 per-core [80,160] PSUM dumps, extract diagonal batch
blocks, S = Minv S~ Minv^T, then the tiny [16,100] weighted MSE.
"""
import numpy as np

T, B, NA, D = 512, 16, 32, 2
KMAX = 10
NCORES = 8
TLOC = T // NCORES          # 64 timesteps per core
KN = KMAX * KMAX

SQRT2 = float(np.sqrt(2.0))

# plane construction DAG (dependency order):
#   ('sin',)            plane = cos(pi x)
#   ('square', src)     plane = (sqrt2 * p_src)^2
#   ('mul', i, j)       plane = p_i * p_j          (DVE TT)
#   ('one',)            plane = 1                  (Pool memset)
# [p3,p5,p7,p9] = [p2,p4,p6,p8]*p1 is emitted as ONE batched TT.
SCHEME = {
    1: ("sin",),
    2: ("square", 1),
    4: ("mul", 2, 2),       # Pool TT when cfg["p4_pool"], else ACT Square
    6: ("mul", 2, 4),
    8: ("mul", 4, 4),
    3: ("mul", 2, 1),
    5: ("mul", 4, 1),
    7: ("mul", 6, 1),
    9: ("mul", 8, 1),
    0: ("one",),
}

_STATE = {}

CFG = {"unroll": 16, "bufs": 6, "pbufs": 6, "skip": (),
       "batched_tt": True, "p4_pool": True, "so": "act", "pe_half": False,
       "preinit_p0": False, "gsplit": False, "ps_onebank": True,
       "dma_out_eng": "sync"}


def _np_constants():
    """Replicates reference._constants() exactly in numpy (L = ones)."""
    L = np.ones(D, dtype=np.float32)
    grids = np.meshgrid(*[np.arange(KMAX) for _ in range(D)], indexing="ij")
    K = np.stack(grids, -1).reshape(-1, D).astype(np.float32)          # [100, 2]
    k_scaled = K * np.pi / L
    nf = np.where(K[:, -1] != 0, np.sqrt(L[-1] / 2.0), 1.0).astype(np.float32)
    nw = ((1.0 + (k_scaled ** 2).sum(-1)) ** (-(D + 1) / 2.0) * 100.0).astype(np.float32)
    safe_k = np.where(K != 0, k_scaled, 1.0)
    term = np.where(K != 0,
                    (np.exp(1j * k_scaled * L) - 1.0) / (1j * safe_k * L),
                    1.0 + 0j)
    cd = (term.prod(-1).real / nf).astype(np.float32)                  # [100]
    return nf, nw, cd


def _build_M():
    """M[k] = stored plane k as a linear combination of c_0..c_9.

    Symbolic expansion over the Chebyshev product rule:
    c_m * c_n = (c_{m+n} + c_{|m-n|}) / 2 (terms beyond c_9 are dropped:
    the construction never produces them for k<=9 products used here).
    """
    def mul(a, b):
        out = np.zeros(KMAX)
        for m in range(KMAX):
            if not a[m]:
                continue
            for n in range(KMAX):
                v = a[m] * b[n]
                if not v:
                    continue
                assert m + n < KMAX
                out[m + n] += v / 2
                out[abs(m - n)] += v / 2
        return out

    M = np.zeros((KMAX, KMAX))
    for k, op in SCHEME.items():
        if op[0] == "sin":
            M[k, 1] = 1.0
        elif op[0] == "square":
            M[k] = mul(M[op[1]] * SQRT2, M[op[1]] * SQRT2)
        elif op[0] == "mul":
            M[k] = mul(M[op[1]], M[op[2]])
        elif op[0] == "one":
            M[k, 0] = 1.0
    return M


def _build(reps: int = 1, loop: bool = False, cfg: dict | None = None):
    import concourse.tile as tile
    from concourse import bacc, mybir
    from concourse.bass import AP

    cfg = {**CFG, **(cfg or {})}
    f32 = mybir.dt.float32
    f16 = mybir.dt.float16
    bf16 = mybir.dt.bfloat16
    AF = mybir.ActivationFunctionType
    OP = mybir.AluOpType

    nc = bacc.Bacc("TRN2", target_bir_lowering=False, debug=False)
    # x is staged host-side as fp16 (x in [0,1): 2^-12 rounding, far below
    # the loss tolerance) to halve the input HBM traffic.
    xx = nc.dram_tensor("xx", [128, 512], f16, kind="ExternalInput").ap()
    sout = nc.dram_tensor("sout", [80, 160], f32, kind="ExternalOutput").ap()

    unroll = cfg["unroll"] if loop else 1
    if loop:
        assert reps % unroll == 0, (reps, unroll)

    with tile.TileContext(nc) as tc:
        with tc.tile_pool(name="cpool", bufs=1) as cpool, \
             tc.tile_pool(name="pool", bufs=cfg["bufs"]) as pool, \
             tc.tile_pool(name="ppool", bufs=cfg["pbufs"],
                          space="PSUM") as ppool:
            scale_t = cpool.tile([128, 1], f32)     # -pi   (Sin input scale)
            bias_t = cpool.tile([128, 1], f32)      # pi/2  (Sin input bias)
            zero_t = cpool.tile([128, 1], f32)      # 0     (Square bias)
            nc.vector.memset(scale_t[:], -float(np.pi))
            nc.vector.memset(zero_t[:], 0.0)
            # warmup: loads the Sin table outside the loop; body activations
            # depend on its (zero) output via their bias tiles, so the
            # table load cannot sink into the loop.
            nc.scalar.activation(zero_t[:], zero_t[:], AF.Sin,
                                 bias=0.0, scale=1.0)
            nc.vector.tensor_scalar(bias_t[:], zero_t[:],
                                    0.5 * float(np.pi), None, OP.add)

            def c_tile():
                C = pool.tile([128, 5120], bf16, tag="C")
                return C, C[:].rearrange("p (q k b) -> p q k b",
                                         q=64, k=KMAX)

            if cfg["preinit_p0"]:
                # plane0 (all-ones) is constant: write it once into every
                # rotating C buffer before the loop. In-loop bodies never
                # touch the k=0 columns, and the For_i replay uses static
                # SBUF addresses, so the pre-init stays valid.
                for _ in range(cfg["bufs"]):
                    _, Cb0 = c_tile()
                    nc.gpsimd.memset(Cb0[:, :, 0], 1.0)

            def emit_planes(Xv, Cv):
                """Build planes 1..9 for one (X view, C view) column slice."""
                # p1 = sin(pi/2 - pi x) = cos(pi x); arg in (-pi/2, pi/2]
                nc.scalar.activation(Cv[:, :, 1], Xv, AF.Sin,
                                     bias=bias_t[:], scale=scale_t[:])
                # p2 = (sqrt2 c1)^2 = 1 + c2
                nc.scalar.activation(Cv[:, :, 2], Cv[:, :, 1], AF.Square,
                                     bias=zero_t[:], scale=SQRT2)
                p4 = Cv[:, :, 4]
                if cfg["p4_pool"]:
                    # p4 = p2^2 on the otherwise-idle Pool engine
                    nc.gpsimd.tensor_tensor(p4, Cv[:, :, 2], Cv[:, :, 2],
                                            OP.mult)
                else:
                    nc.scalar.activation(p4, Cv[:, :, 2], AF.Square,
                                         bias=zero_t[:], scale=SQRT2)
                if cfg["batched_tt"]:
                    # [p6,p8] = broadcast(p4) * [p2,p4]   (one DVE TT)
                    bc4 = AP(p4.tensor, p4.offset,
                             [p4.ap[0], p4.ap[1], [0, 2], p4.ap[2]])
                    nc.vector.tensor_tensor(Cv[:, :, 6:KMAX:2],
                                            bc4, Cv[:, :, 2:6:2], OP.mult)
                    # [p3,p5,p7,p9] = [p2,p4,p6,p8] * broadcast(p1)
                    p1 = Cv[:, :, 1]
                    bc1 = AP(p1.tensor, p1.offset,
                             [p1.ap[0], p1.ap[1], [0, 4], p1.ap[2]])
                    nc.vector.tensor_tensor(Cv[:, :, 3:KMAX:2],
                                            Cv[:, :, 2:KMAX:2], bc1, OP.mult)
                else:
                    nc.vector.tensor_tensor(Cv[:, :, 6], Cv[:, :, 2], p4,
                                            OP.mult)
                    nc.vector.tensor_tensor(Cv[:, :, 8], p4, p4, OP.mult)
                    for k, i in ((3, 2), (5, 4), (7, 6), (9, 8)):
                        nc.vector.tensor_tensor(Cv[:, :, k], Cv[:, :, i],
                                                Cv[:, :, 1], OP.mult)

            def body(_i=None):
                XX = pool.tile([128, 512], f16, tag="XX")
                C, Cb = c_tile()
                # layouts (q = dd*32 + c*2 + g indexes the 64 matmul blocks):
                #   XX col = q*8 + b'            (= dd*256 + c*16 + g*8 + b')
                #   C  col = q*80 + k*8 + b'     (k: mode plane)
                # every matmul operand C[:, q*80 : q*80+80] is contiguous.
                skip = cfg["skip"]
                nc.sync.dma_start(XX[:], xx)
                if "act" not in skip and "tt" not in skip:
                    if cfg["gsplit"]:
                        # per batch-group halves (q even / q odd): the 16
                        # g=0 matmuls only depend on the g=0 half, so PE
                        # starts earlier in each body's pipeline.
                        Xg = XX[:].rearrange("p (qq g b) -> p qq g b", g=2, b=8)
                        Cg = C[:].rearrange("p (qq g k b) -> p qq g k b",
                                            g=2, k=KMAX, b=8)
                        for g in range(2):
                            emit_planes(Xg[:, :, g], Cg[:, :, g])
                    else:
                        Xb = XX[:].rearrange("p (q b) -> p q b", q=64)
                        emit_planes(Xb, Cb)
                if "pool" not in skip and not cfg["preinit_p0"]:
                    nc.gpsimd.memset(Cb[:, :, 0], 1.0)

                if cfg["ps_onebank"]:
                    # BOTH accumulation groups in ONE PSUM bank ([80,160]
                    # f32 = 640B/partition fits the 2KB bank): the g=0
                    # start=True matmul zeroes the whole bank (PSUM zeroing
                    # is bank-granular), and the in-order PE runs it before
                    # g=1's first start=False accumulate. skip_group_check
                    # bypasses the framework's one-group-per-zero-region
                    # guard, which rejects exactly this (safe) pattern.
                    pst = ppool.tile([80, 160], f32, name="ps", tag="ps")
                    ps = [pst[:, 0:80], pst[:, 80:160]]
                    starts = {0: (True, False), 1: (False, False)}
                else:
                    pst = None
                    tiles = [ppool.tile([80, 80], f32, name=f"ps{g}",
                                        tag=f"ps{g}") for g in range(2)]
                    ps = [t[:] for t in tiles]
                    starts = {0: (True, True)}
                if "pe" not in skip:
                    ngroups = 1 if cfg.get("pe_half") else 2
                    for c in range(16):
                        for g in range(ngroups):
                            q0 = c * 2 + g
                            st = starts.get(c, (False, False))[g]
                            nc.tensor.matmul(
                                ps[g],
                                C[:, q0 * 80:q0 * 80 + 80],
                                C[:, (32 + q0) * 80:(32 + q0) * 80 + 80],
                                start=st, stop=(c == 15),
                                skip_group_check=cfg["ps_onebank"])
                    if ngroups == 1:
                        nc.tensor.matmul(ps[1], C[:, 0:80], C[:, 0:80],
                                         start=not cfg["ps_onebank"],
                                         stop=True,
                                         skip_group_check=cfg["ps_onebank"])
                if "out" not in skip:
                    SO = pool.tile([80, 160], f32, tag="SO")
                    cp = (nc.vector.tensor_copy if cfg["so"] == "dve"
                          else nc.scalar.copy)
                    if cfg["ps_onebank"]:
                        cp(SO[0:80, :], pst[:])
                    else:
                        cp(SO[0:80, 0:80], ps[0])
                        cp(SO[0:80, 80:160], ps[1])
                    # out-DMA engine is configurable: issuing from ACT uses
                    # a different HWDGE queue than the SP-issued input DMA
                    oeng = {"sync": nc.sync, "act": nc.scalar,
                            "dve": nc.vector}[cfg["dma_out_eng"]]
                    oeng.dma_start(sout, SO[0:80, :])

            if loop:
                with tc.For_i(0, reps // unroll, 1) as i:
                    for _ in range(unroll):
                        body(i)
            else:
                for _ in range(reps):
                    body()

    nc.compile()
    return nc


def _get_state():
    if "nc" not in _STATE:
        _STATE["nc"] = _build()
    return _STATE["nc"]


def _shard_inputs(x: np.ndarray):
    """x [512, 16, 32, 2] -> per-core {xx [128, 512]}.

    xx free layout: dd*256 + c*16 + g*8 + b'  (q-blocks ordered (c, g);
    batch b = g*8 + b'). partition p = tp*32 + a for timestep-subgroup
    tp in 4, agent a in 32.
    """
    in_maps = []
    for core in range(NCORES):
        xc = x[core * TLOC:(core + 1) * TLOC]      # [64, 16, 32, 2]
        arr = xc.reshape(16, 4, 2, 8, 32, 2)       # (c, tp, g, b', a, d)
        arr = arr.transpose(5, 1, 4, 0, 2, 3)      # (d, tp, a, c, g, b')
        arr = arr.reshape(2, 128, 256)
        xxc = np.concatenate([arr[0], arr[1]], axis=1)
        in_maps.append({"xx": np.ascontiguousarray(xxc).astype(np.float16)})
    return in_maps


def _gather(souts):
    """souts: list of 8 [80, 160] partials -> scalar loss (float32).

    sout row = k0*8 + b', col (80*g + k1*8 + b'') for batch b = 8*g + b'.
    """
    total = np.zeros((80, 160), dtype=np.float64)
    for s in souts:
        total += s.astype(np.float64)
    St = np.empty((B, KMAX, KMAX), dtype=np.float64)
    for g in range(2):
        for bp in range(8):
            St[8 * g + bp] = total[bp::8, 80 * g + bp:80 * (g + 1):8]
    Minv = np.linalg.inv(_build_M())
    S = np.einsum("ki,bij,lj->bkl", Minv, St, Minv)
    nf, nw, cd = _np_constants()
    coeffs = S.reshape(B, KN) / (NA * T) / nf[None, :].astype(np.float64)
    d = nw[None, :].astype(np.float64) * (coeffs - cd[None, :].astype(np.float64))
    loss = np.mean(d * d)
    return np.float32(loss)


def kernel(x: np.ndarray) -> np.ndarray:
    from concourse.bass_utils import run_bass_kernel_spmd

    nc = _get_state()
    in_maps = _shard_inputs(np.asarray(x, dtype=np.float32))
    res = run_bass_kernel_spmd(nc, in_maps, list(range(NCORES)))
    souts = [r["sout"] for r in res.results]
    return _gather(souts)


# revision 35
# speedup vs baseline: 1.1084x; 1.1084x over previous
"""Trainium2 Bass kernel for the ergodicity loss (Chebyshev-cascade scheme).

Math: for x[T=512, B=16, N=32, d=2] in [0,1]^2 and modes (k0,k1) in {0..9}^2:
    basis = cos(pi*k0*x0) * cos(pi*k1*x1)                    (separable)
    coeffs[b, k0, k1] = sum_{t,n} basis / (T*N) / nf[k1]
    loss = mean((nw * (coeffs - cd))**2)

Device strategy (8 cores, data-parallel over T: 64 timesteps/core):
  Per core: 2048 points per batch as 128 partitions x 16 chunks, both
  coordinate dims side by side (xx[p, dd*256 + c*16 + b]).

  x is staged host-side as fp16 (x in [0,1): 2^-12 rounding error, far
  below the loss tolerance) to halve the input HBM traffic.

  Mode columns (bf16) hold, per mode k, a KNOWN LINEAR COMBINATION of
  the true cosines c_j = cos(pi*j*x), built WITHOUT any range-reduction
  tricks via a Chebyshev product cascade seeded by one direct Sin:
    - p1 = Sin(-pi*x + pi/2) = cos(pi*x) = c1   (arg in (-pi/2, pi/2],
      inside the HW Sin table domain; ACT, reads xx fp16 directly)
    - p2 = Square(sqrt2*p1) = 2*c1^2 = 1 + c2             (ACT)
    - p4 = p2*p2                                          (Pool TT)
    - [p6,p8] = broadcast(p4) * [p2,p4]                   (one DVE TT)
    - [p3,p5,p7,p9] = [p2,p4,p6,p8] * broadcast(p1)       (one DVE TT)
      (stride-16 plane APs + stride-0 broadcast sub-columns)
    - p0 = 1                                              (Pool memset)
  The mode-k columns are thus M @ c for a constant triangular M
  (|Minv|max ~ 100; high-mode noise amplification is crushed by the
  k^-3 loss weights nw); the PSUM Gram matrix S~ = M S M^T is unmixed
  on the host with M^-1 (exact linear algebra, fp64).

  PE: per (chunk c in 16, batch-group g in 2): lhsT = dim-0 columns
  [k:10 x b:8] (contiguous 80-col block), rhs = dim-1 columns;
  accumulate into one PSUM[80, 160] tile over the 16 chunks — BOTH
  groups share one 2KB bank (g=0's start=True zeroes the whole bank
  before g=1's first start=False accumulate; skip_group_check bypasses
  the one-group-per-bank guard), 6-deep rotation. Off-diagonal batch
  blocks unused. One ACT copy evacuates PSUM to SBUF, one DMA out.

  The activation-table load (1283 ns) is hoisted out of the bench loop
  by a pre-loop warmup Sin whose (zero) output feeds the bias tile
  every in-loop activation consumes (Sin/Square/Copy share one HW
  activation table set, so no in-loop reloads occur).
Host: sum 8 per-core [80,160] PSUM dumps, extract diagonal batch
blocks, S = Minv S~ Minv^T, then the tiny [16,100] weighted MSE.
"""
import numpy as np

T, B, NA, D = 512, 16, 32, 2
KMAX = 10
NCORES = 8
TLOC = T // NCORES          # 64 timesteps per core
KN = KMAX * KMAX

SQRT2 = float(np.sqrt(2.0))

# plane construction DAG (dependency order):
#   ('sin',)            plane = cos(pi x)
#   ('square', src)     plane = (sqrt2 * p_src)^2
#   ('mul', i, j)       plane = p_i * p_j          (DVE TT)
#   ('one',)            plane = 1                  (Pool memset)
# [p3,p5,p7,p9] = [p2,p4,p6,p8]*p1 is emitted as ONE batched TT.
SCHEME = {
    1: ("sin",),
    2: ("square", 1),
    4: ("mul", 2, 2),       # Pool TT when cfg["p4_pool"], else ACT Square
    6: ("mul", 2, 4),
    8: ("mul", 4, 4),
    3: ("mul", 2, 1),
    5: ("mul", 4, 1),
    7: ("mul", 6, 1),
    9: ("mul", 8, 1),
    0: ("one",),
}

_STATE = {}

CFG = {"unroll": 16, "bufs": 6, "pbufs": 6, "skip": (),
       "batched_tt": True, "p4_pool": True, "so": "act", "pe_half": False,
       "preinit_p0": False, "gsplit": False, "ps_onebank": True,
       "dma_out_eng": "sync", "out_delay": 4, "copy_delay": 0}


def _np_constants():
    """Replicates reference._constants() exactly in numpy (L = ones)."""
    L = np.ones(D, dtype=np.float32)
    grids = np.meshgrid(*[np.arange(KMAX) for _ in range(D)], indexing="ij")
    K = np.stack(grids, -1).reshape(-1, D).astype(np.float32)          # [100, 2]
    k_scaled = K * np.pi / L
    nf = np.where(K[:, -1] != 0, np.sqrt(L[-1] / 2.0), 1.0).astype(np.float32)
    nw = ((1.0 + (k_scaled ** 2).sum(-1)) ** (-(D + 1) / 2.0) * 100.0).astype(np.float32)
    safe_k = np.where(K != 0, k_scaled, 1.0)
    term = np.where(K != 0,
                    (np.exp(1j * k_scaled * L) - 1.0) / (1j * safe_k * L),
                    1.0 + 0j)
    cd = (term.prod(-1).real / nf).astype(np.float32)                  # [100]
    return nf, nw, cd


def _build_M():
    """M[k] = stored plane k as a linear combination of c_0..c_9.

    Symbolic expansion over the Chebyshev product rule:
    c_m * c_n = (c_{m+n} + c_{|m-n|}) / 2 (terms beyond c_9 are dropped:
    the construction never produces them for k<=9 products used here).
    """
    def mul(a, b):
        out = np.zeros(KMAX)
        for m in range(KMAX):
            if not a[m]:
                continue
            for n in range(KMAX):
                v = a[m] * b[n]
                if not v:
                    continue
                assert m + n < KMAX
                out[m + n] += v / 2
                out[abs(m - n)] += v / 2
        return out

    M = np.zeros((KMAX, KMAX))
    for k, op in SCHEME.items():
        if op[0] == "sin":
            M[k, 1] = 1.0
        elif op[0] == "square":
            M[k] = mul(M[op[1]] * SQRT2, M[op[1]] * SQRT2)
        elif op[0] == "mul":
            M[k] = mul(M[op[1]], M[op[2]])
        elif op[0] == "one":
            M[k, 0] = 1.0
    return M


def _build(reps: int = 1, loop: bool = False, cfg: dict | None = None):
    import concourse.tile as tile
    from concourse import bacc, mybir
    from concourse.bass import AP

    cfg = {**CFG, **(cfg or {})}
    f32 = mybir.dt.float32
    f16 = mybir.dt.float16
    bf16 = mybir.dt.bfloat16
    AF = mybir.ActivationFunctionType
    OP = mybir.AluOpType

    nc = bacc.Bacc("TRN2", target_bir_lowering=False, debug=False)
    # x is staged host-side as fp16 (x in [0,1): 2^-12 rounding, far below
    # the loss tolerance) to halve the input HBM traffic.
    xx = nc.dram_tensor("xx", [128, 512], f16, kind="ExternalInput").ap()
    sout = nc.dram_tensor("sout", [80, 160], f32, kind="ExternalOutput").ap()

    unroll = cfg["unroll"] if loop else 1
    if loop:
        assert reps % unroll == 0, (reps, unroll)

    with tile.TileContext(nc) as tc:
        with tc.tile_pool(name="cpool", bufs=1) as cpool, \
             tc.tile_pool(name="pool", bufs=cfg["bufs"]) as pool, \
             tc.tile_pool(name="ppool", bufs=cfg["pbufs"],
                          space="PSUM") as ppool:
            scale_t = cpool.tile([128, 1], f32)     # -pi   (Sin input scale)
            bias_t = cpool.tile([128, 1], f32)      # pi/2  (Sin input bias)
            zero_t = cpool.tile([128, 1], f32)      # 0     (Square bias)
            nc.vector.memset(scale_t[:], -float(np.pi))
            nc.vector.memset(zero_t[:], 0.0)
            # warmup: loads the Sin table outside the loop; body activations
            # depend on its (zero) output via their bias tiles, so the
            # table load cannot sink into the loop.
            nc.scalar.activation(zero_t[:], zero_t[:], AF.Sin,
                                 bias=0.0, scale=1.0)
            nc.vector.tensor_scalar(bias_t[:], zero_t[:],
                                    0.5 * float(np.pi), None, OP.add)

            def c_tile():
                C = pool.tile([128, 5120], bf16, tag="C")
                return C, C[:].rearrange("p (q k b) -> p q k b",
                                         q=64, k=KMAX)

            if cfg["preinit_p0"]:
                # plane0 (all-ones) is constant: write it once into every
                # rotating C buffer before the loop. In-loop bodies never
                # touch the k=0 columns, and the For_i replay uses static
                # SBUF addresses, so the pre-init stays valid.
                for _ in range(cfg["bufs"]):
                    _, Cb0 = c_tile()
                    nc.gpsimd.memset(Cb0[:, :, 0], 1.0)

            def emit_planes(Xv, Cv):
                """Build planes 1..9 for one (X view, C view) column slice."""
                # p1 = sin(pi/2 - pi x) = cos(pi x); arg in (-pi/2, pi/2]
                nc.scalar.activation(Cv[:, :, 1], Xv, AF.Sin,
                                     bias=bias_t[:], scale=scale_t[:])
                # p2 = (sqrt2 c1)^2 = 1 + c2
                nc.scalar.activation(Cv[:, :, 2], Cv[:, :, 1], AF.Square,
                                     bias=zero_t[:], scale=SQRT2)
                p4 = Cv[:, :, 4]
                if cfg["p4_pool"]:
                    # p4 = p2^2 on the otherwise-idle Pool engine
                    nc.gpsimd.tensor_tensor(p4, Cv[:, :, 2], Cv[:, :, 2],
                                            OP.mult)
                else:
                    nc.scalar.activation(p4, Cv[:, :, 2], AF.Square,
                                         bias=zero_t[:], scale=SQRT2)
                if cfg["batched_tt"]:
                    # [p6,p8] = broadcast(p4) * [p2,p4]   (one DVE TT)
                    bc4 = AP(p4.tensor, p4.offset,
                             [p4.ap[0], p4.ap[1], [0, 2], p4.ap[2]])
                    nc.vector.tensor_tensor(Cv[:, :, 6:KMAX:2],
                                            bc4, Cv[:, :, 2:6:2], OP.mult)
                    # [p3,p5,p7,p9] = [p2,p4,p6,p8] * broadcast(p1)
                    p1 = Cv[:, :, 1]
                    bc1 = AP(p1.tensor, p1.offset,
                             [p1.ap[0], p1.ap[1], [0, 4], p1.ap[2]])
                    nc.vector.tensor_tensor(Cv[:, :, 3:KMAX:2],
                                            Cv[:, :, 2:KMAX:2], bc1, OP.mult)
                else:
                    nc.vector.tensor_tensor(Cv[:, :, 6], Cv[:, :, 2], p4,
                                            OP.mult)
                    nc.vector.tensor_tensor(Cv[:, :, 8], p4, p4, OP.mult)
                    for k, i in ((3, 2), (5, 4), (7, 6), (9, 8)):
                        nc.vector.tensor_tensor(Cv[:, :, k], Cv[:, :, i],
                                                Cv[:, :, 1], OP.mult)

            def body(_i=None):
                XX = pool.tile([128, 512], f16, tag="XX")
                C, Cb = c_tile()
                # layouts (q = dd*32 + c*2 + g indexes the 64 matmul blocks):
                #   XX col = q*8 + b'            (= dd*256 + c*16 + g*8 + b')
                #   C  col = q*80 + k*8 + b'     (k: mode plane)
                # every matmul operand C[:, q*80 : q*80+80] is contiguous.
                skip = cfg["skip"]
                nc.sync.dma_start(XX[:], xx)
                if "act" not in skip and "tt" not in skip:
                    if cfg["gsplit"]:
                        # per batch-group halves (q even / q odd): the 16
                        # g=0 matmuls only depend on the g=0 half, so PE
                        # starts earlier in each body's pipeline.
                        Xg = XX[:].rearrange("p (qq g b) -> p qq g b", g=2, b=8)
                        Cg = C[:].rearrange("p (qq g k b) -> p qq g k b",
                                            g=2, k=KMAX, b=8)
                        for g in range(2):
                            emit_planes(Xg[:, :, g], Cg[:, :, g])
                    else:
                        Xb = XX[:].rearrange("p (q b) -> p q b", q=64)
                        emit_planes(Xb, Cb)
                if "pool" not in skip and not cfg["preinit_p0"]:
                    nc.gpsimd.memset(Cb[:, :, 0], 1.0)

                if cfg["ps_onebank"]:
                    # BOTH accumulation groups in ONE PSUM bank ([80,160]
                    # f32 = 640B/partition fits the 2KB bank): the g=0
                    # start=True matmul zeroes the whole bank (PSUM zeroing
                    # is bank-granular), and the in-order PE runs it before
                    # g=1's first start=False accumulate. skip_group_check
                    # bypasses the framework's one-group-per-zero-region
                    # guard, which rejects exactly this (safe) pattern.
                    pst = ppool.tile([80, 160], f32, name="ps", tag="ps")
                    ps = [pst[:, 0:80], pst[:, 80:160]]
                    starts = {0: (True, False), 1: (False, False)}
                else:
                    pst = None
                    tiles = [ppool.tile([80, 80], f32, name=f"ps{g}",
                                        tag=f"ps{g}") for g in range(2)]
                    ps = [t[:] for t in tiles]
                    starts = {0: (True, True)}
                if "pe" not in skip:
                    ngroups = 1 if cfg.get("pe_half") else 2
                    for c in range(16):
                        for g in range(ngroups):
                            q0 = c * 2 + g
                            st = starts.get(c, (False, False))[g]
                            nc.tensor.matmul(
                                ps[g],
                                C[:, q0 * 80:q0 * 80 + 80],
                                C[:, (32 + q0) * 80:(32 + q0) * 80 + 80],
                                start=st, stop=(c == 15),
                                skip_group_check=cfg["ps_onebank"])
                    if ngroups == 1:
                        nc.tensor.matmul(ps[1], C[:, 0:80], C[:, 0:80],
                                         start=not cfg["ps_onebank"],
                                         stop=True,
                                         skip_group_check=cfg["ps_onebank"])
                if "out" in skip:
                    return None
                return pst if cfg["ps_onebank"] else tuple(ps)

            def emit_copy(pst):
                # PSUM -> SBUF evacuation; delayed by cfg["copy_delay"]
                # bodies so its PE-stop wait is already resolved when it
                # reaches ACT's in-order engine queue (otherwise it blocks
                # the next body's Sin until PE finishes this body).
                SO = pool.tile([80, 160], f32, tag="SO")
                cp = (nc.vector.tensor_copy if cfg["so"] == "dve"
                      else nc.scalar.copy)
                if isinstance(pst, tuple):
                    cp(SO[0:80, 0:80], pst[0])
                    cp(SO[0:80, 80:160], pst[1])
                else:
                    cp(SO[0:80, :], pst[:])
                return SO

            def emit_out_dma(SO):
                # out-DMA engine is configurable: issuing from ACT uses
                # a different HWDGE queue than the SP-issued input DMA
                oeng = {"sync": nc.sync, "act": nc.scalar,
                        "dve": nc.vector}[cfg["dma_out_eng"]]
                oeng.dma_start(sout, SO[0:80, :])

            if loop:
                # software-pipeline the out-DMA by cfg["out_delay"] bodies:
                # SP's sequencer is in-order and a DMA holds SEQ while its
                # sem waits resolve, so emitting body j's PE-gated out-DMA
                # after body j+delay's input DMA keeps the input prefetch
                # running ahead of PE instead of serializing behind it.
                # Each iteration still issues one in- and one out-DMA per
                # body (the tail flushes); only the interleaving changes.
                kc = cfg["copy_delay"]
                kd = max(cfg["out_delay"] - kc, 0)
                with tc.For_i(0, reps // unroll, 1) as i:
                    ps_q, so_q = [], []

                    def flush_one(queue, limit, emit):
                        if len(queue) > limit:
                            v = queue.pop(0)
                            return emit(v) if v is not None else None
                        return None

                    for _ in range(unroll):
                        ps_q.append(body(i))
                        so_q.append(flush_one(ps_q, kc, emit_copy))
                        flush_one(so_q, kd, emit_out_dma)
                    for v in ps_q:
                        so_q.append(emit_copy(v) if v is not None else None)
                    for v in so_q:
                        if v is not None:
                            emit_out_dma(v)
            else:
                for _ in range(reps):
                    v = body()
                    if v is not None:
                        emit_out_dma(emit_copy(v))

    nc.compile()
    return nc


def _get_state():
    if "nc" not in _STATE:
        _STATE["nc"] = _build()
    return _STATE["nc"]


def _shard_inputs(x: np.ndarray):
    """x [512, 16, 32, 2] -> per-core {xx [128, 512]}.

    xx free layout: dd*256 + c*16 + g*8 + b'  (q-blocks ordered (c, g);
    batch b = g*8 + b'). partition p = tp*32 + a for timestep-subgroup
    tp in 4, agent a in 32.
    """
    in_maps = []
    for core in range(NCORES):
        xc = x[core * TLOC:(core + 1) * TLOC]      # [64, 16, 32, 2]
        arr = xc.reshape(16, 4, 2, 8, 32, 2)       # (c, tp, g, b', a, d)
        arr = arr.transpose(5, 1, 4, 0, 2, 3)      # (d, tp, a, c, g, b')
        arr = arr.reshape(2, 128, 256)
        xxc = np.concatenate([arr[0], arr[1]], axis=1)
        in_maps.append({"xx": np.ascontiguousarray(xxc).astype(np.float16)})
    return in_maps


def _gather(souts):
    """souts: list of 8 [80, 160] partials -> scalar loss (float32).

    sout row = k0*8 + b', col (80*g + k1*8 + b'') for batch b = 8*g + b'.
    """
    total = np.zeros((80, 160), dtype=np.float64)
    for s in souts:
        total += s.astype(np.float64)
    St = np.empty((B, KMAX, KMAX), dtype=np.float64)
    for g in range(2):
        for bp in range(8):
            St[8 * g + bp] = total[bp::8, 80 * g + bp:80 * (g + 1):8]
    Minv = np.linalg.inv(_build_M())
    S = np.einsum("ki,bij,lj->bkl", Minv, St, Minv)
    nf, nw, cd = _np_constants()
    coeffs = S.reshape(B, KN) / (NA * T) / nf[None, :].astype(np.float64)
    d = nw[None, :].astype(np.float64) * (coeffs - cd[None, :].astype(np.float64))
    loss = np.mean(d * d)
    return np.float32(loss)


def kernel(x: np.ndarray) -> np.ndarray:
    from concourse.bass_utils import run_bass_kernel_spmd

    nc = _get_state()
    in_maps = _shard_inputs(np.asarray(x, dtype=np.float32))
    res = run_bass_kernel_spmd(nc, in_maps, list(range(NCORES)))
    souts = [r["sout"] for r in res.results]
    return _gather(souts)
